# revision 1
# baseline (speedup 1.0000x reference)
"""Trainium2 Bass kernel for the ButterflyModule problem.

Semantics (N=4096 rows, B=8192 cols):
  x = data[indices_in]
  4 Givens-rotation butterfly layers (strides 1,2,4,8 within 16-row blocks)
  bias + smooth-ReLU on rows with (row%16)<8
  4 more butterfly layers (strides 1,2,4,8)
  out = data with rows idx_out replaced by the result

Device strategy: the 4 input layers compose into a dense 16x16 matrix per
16-row block (256 blocks), same for the 4 output layers.  Each 128-row group
is then one block-diagonal 128x128 matmul on the TensorEngine.  The
activation folds into per-partition scalars:

  y' = D.Min @ x + D.b          (D = diag(0.5 on act rows, 1 elsewhere))
  u  = m * y'                   (m = 1 on act rows, 0 elsewhere; ACT Square scale)
  s  = sqrt(u^2 + (0.05)^2 * m) (ACT Sqrt with per-partition bias)
  z  = y' + s                   (act rows: 0.5*(xa+sqrt(xa^2+0.01)); else y)
  out = Mout @ z

Rows are sharded across the 8 cores (512 rows each); rotations never cross
16-row block boundaries so there is no cross-core communication.
"""

import sys

if "/opt/trn_rl_repo" not in sys.path:
    sys.path.insert(0, "/opt/trn_rl_repo")

import numpy as np

N_ROWS = 4096
N_COLS = 8192
COL_BLOCK = 16
NUM_ACT = 8
CURVATURE = 0.1
N_CORES = 8
ROWS_PER_CORE = N_ROWS // N_CORES          # 512
GROUPS_PER_CORE = ROWS_PER_CORE // 128     # 4
FREE = 512                                 # matmul moving-dim tile (fp32 max)
N_FTILES = N_COLS // FREE                  # 16

_PROGRAM_CACHE = {}


def _butterfly_mats(angles64):
    """Compose butterfly layers into per-block 16x16 matrices.

    angles64: [8, 2048] float64.  Returns (Min, Mout) each [256, 16, 16],
    where layer l uses stride 1<<(l%4) and block b uses angles[l, 8b:8b+8]
    ordered by the low row index within the block.
    """
    nb = N_ROWS // COL_BLOCK

    def accum(l0, l1):
        G = np.broadcast_to(np.eye(COL_BLOCK), (nb, COL_BLOCK, COL_BLOCK)).copy()
        for l in range(l0, l1):
            stride = 1 << (l % 4)
            offs = [o for o in range(COL_BLOCK) if (o & stride) == 0]
            a = angles64[l].reshape(nb, NUM_ACT)
            c = np.cos(a)
            s = np.sin(a)
            for k, o in enumerate(offs):
                gl = G[:, o, :].copy()
                gh = G[:, o + stride, :].copy()
                G[:, o, :] = c[:, k, None] * gl + s[:, k, None] * gh
                G[:, o + stride, :] = -s[:, k, None] * gl + c[:, k, None] * gh
        return G

    return accum(0, 4), accum(4, 8)


def _host_weights(angles, biases):
    """Build per-core weight tensors for the device kernel."""
    ang64 = np.asarray(angles, np.float64)
    b64 = np.asarray(biases, np.float64)
    Min, Mout = _butterfly_mats(ang64)

    off = np.arange(COL_BLOCK)
    d16 = np.where(off < NUM_ACT, 0.5, 1.0)          # post-layer-4 scale
    Minp = Min * d16[None, :, None]                  # diag(d) @ Min (row scale)

    MinT = Minp.transpose(0, 2, 1)                   # per-block lhsT
    MoutT = Mout.transpose(0, 2, 1)

    def block_diag8(blocksT):
        out = np.zeros((128, 128))
        for i in range(8):
            out[i * 16:(i + 1) * 16, i * 16:(i + 1) * 16] = blocksT[i]
        return out

    # bias' = d * bias on act rows, 0 elsewhere, laid out over all 4096 rows
    bf = np.zeros(N_ROWS)
    blk = np.arange(N_ROWS // COL_BLOCK)
    for o in range(NUM_ACT):
        bf[blk * COL_BLOCK + o] = 0.5 * b64[blk * NUM_ACT + o]

    p = np.arange(128)
    act = (p % COL_BLOCK) < NUM_ACT
    m_vec = np.where(act, 1.0, 0.0).astype(np.float32).reshape(128, 1)
    sqb_vec = np.where(act, (0.5 * CURVATURE) ** 2, 0.0).astype(np.float32)
    sqb_vec = sqb_vec.reshape(128, 1)

    per_core = []
    for c in range(N_CORES):
        win = np.zeros((GROUPS_PER_CORE, 128, 128))
        wout = np.zeros((GROUPS_PER_CORE, 128, 128))
        for g in range(GROUPS_PER_CORE):
            g_glob = c * GROUPS_PER_CORE + g
            win[g] = block_diag8(MinT[g_glob * 8:(g_glob + 1) * 8])
            wout[g] = block_diag8(MoutT[g_glob * 8:(g_glob + 1) * 8])
        win_dram = win.transpose(1, 0, 2).reshape(128, GROUPS_PER_CORE * 128)
        wout_dram = wout.transpose(1, 0, 2).reshape(128, GROUPS_PER_CORE * 128)
        bias_dram = (
            bf[c * ROWS_PER_CORE:(c + 1) * ROWS_PER_CORE]
            .reshape(GROUPS_PER_CORE, 128)
            .T
        )
        per_core.append(
            {
                "win": np.ascontiguousarray(win_dram, dtype=np.float32),
                "wout": np.ascontiguousarray(wout_dram, dtype=np.float32),
                "biasv": np.ascontiguousarray(bias_dram, dtype=np.float32),
                "mvec": m_vec,
                "sqbv": sqb_vec,
            }
        )
    return per_core


def _build_program(reps=None, use_f32r=None, mode=None, xbufs=None,
                   wbufs=None, pipelined=None):
    import os

    import concourse.bacc as bacc
    import concourse.mybir as mybir
    from concourse.tile import TileContext

    f32 = mybir.dt.float32
    f32r = mybir.dt.float32r
    AFT = mybir.ActivationFunctionType
    Alu = mybir.AluOpType
    if use_f32r is None:
        use_f32r = os.environ.get("BUTTERFLY_FP32R", "0") == "1"
    if reps is None:
        reps = int(os.environ.get("BUTTERFLY_REPS", "1"))
    if mode is None:
        mode = os.environ.get("BUTTERFLY_MODE", "full")  # full|dma|compute
    if xbufs is None:
        xbufs = int(os.environ.get("BUTTERFLY_XBUFS", "3"))
    if wbufs is None:
        wbufs = int(os.environ.get("BUTTERFLY_WBUFS", "4"))
    if pipelined is None:
        pipelined = os.environ.get("BUTTERFLY_PIPE", "1") == "1"
    pybufs = int(os.environ.get("BUTTERFLY_PYBUFS", "2"))
    pobufs = int(os.environ.get("BUTTERFLY_POBUFS", "2"))
    odma = os.environ.get("BUTTERFLY_ODMA", "sp")  # sp | act | pool
    wtile = int(os.environ.get("BUTTERFLY_W", "1024"))
    interpose = os.environ.get("BUTTERFLY_INTERPOSE", "0") == "1"
    xw = int(os.environ.get("BUTTERFLY_XW", "1024"))  # x-load chunk width

    W = 1024                    # megatile width (2 PSUM banks)
    n_wtiles = N_COLS // W      # 8 per row-group

    fmm = f32r if use_f32r else f32

    def mm_cast(ap):
        return ap

    nc = bacc.Bacc("TRN2", target_bir_lowering=False)
    x = nc.dram_tensor("x", [ROWS_PER_CORE, N_COLS], fmm, kind="ExternalInput")
    win = nc.dram_tensor("win", [128, GROUPS_PER_CORE * 128], fmm,
                         kind="ExternalInput")
    wout = nc.dram_tensor("wout", [128, GROUPS_PER_CORE * 128], fmm,
                          kind="ExternalInput")
    biasv = nc.dram_tensor("biasv", [128, GROUPS_PER_CORE], f32,
                           kind="ExternalInput")
    mvec = nc.dram_tensor("mvec", [128, 1], f32, kind="ExternalInput")
    sqbv = nc.dram_tensor("sqbv", [128, 1], f32, kind="ExternalInput")
    yout = nc.dram_tensor("yout", [ROWS_PER_CORE, N_COLS], f32,
                          kind="ExternalOutput")

    with TileContext(nc) as tc:
        with (
            tc.tile_pool(name="consts", bufs=1) as cpool,
            tc.tile_pool(name="xin", bufs=xbufs) as xpool,
            tc.tile_pool(name="work", bufs=wbufs) as wpool,
            tc.tile_pool(name="psum_y", bufs=pybufs, space="PSUM") as pypool,
            tc.tile_pool(name="psum_o", bufs=pobufs, space="PSUM") as popool,
        ):
            win_sb = cpool.tile([128, GROUPS_PER_CORE * 128], fmm)
            wout_sb = cpool.tile([128, GROUPS_PER_CORE * 128], fmm)
            bias_sb = cpool.tile([128, GROUPS_PER_CORE], f32)
            m_sb = cpool.tile([128, 1], f32)
            sqb_sb = cpool.tile([128, 1], f32)
            # group-0 weights first so the first matmul can start early;
            # remaining groups stream in behind the first x tiles.
            g0 = slice(0, 128)
            nc.sync.dma_start(win_sb[:, g0], win[:, g0])
            nc.sync.dma_start(wout_sb[:, g0], wout[:, g0])
            nc.sync.dma_start(bias_sb[:], biasv[:])
            nc.sync.dma_start(m_sb[:], mvec[:])
            nc.sync.dma_start(sqb_sb[:], sqbv[:])
            grest = slice(128, GROUPS_PER_CORE * 128)
            nc.scalar.dma_start(win_sb[:, grest], win[:, grest])
            nc.scalar.dma_start(wout_sb[:, grest], wout[:, grest])

            import contextlib

            stag = os.environ.get("BUTTERFLY_STAG", "0") == "1"
            loop_cm = (tc.For_i(0, reps, 1, staggered_reset=stag)
                       if reps > 1 else contextlib.nullcontext())
            with loop_cm:
                if mode == "full":
                    _emit_body(nc, tc, mybir, x, yout, win_sb, wout_sb,
                               bias_sb, m_sb, sqb_sb, xpool, wpool, pypool,
                               popool, fmm, pipelined=pipelined, odma=odma,
                               W=wtile, interpose=interpose, XW=xw)
                elif mode == "tiny":
                    xt = xpool.tile([128, 1024], fmm, name="xt")
                    nc.sync.dma_start(xt[:], x[0:128, 0:1024])
                    nc.sync.dma_start(yout[0:128, 0:1024], xt[:])
                elif mode == "dma":
                    W = 1024
                    for g in range(GROUPS_PER_CORE):
                        rows = slice(g * 128, (g + 1) * 128)
                        for j in range(N_COLS // W):
                            cols = slice(j * W, (j + 1) * W)
                            xt = xpool.tile([128, W], fmm, name="xt")
                            nc.sync.dma_start(xt[:], x[rows, cols])
                            nc.sync.dma_start(yout[rows, cols], xt[:])
                elif mode == "dmaflat":
                    xf = x[:].flatten().rearrange(
                        "(n p c) -> n p c", p=128, c=1024)
                    yf = yout[:].flatten().rearrange(
                        "(n p c) -> n p c", p=128, c=1024)
                    for i in range(xf.shape[0]):
                        xt = xpool.tile([128, 1024], fmm, name="xt")
                        nc.sync.dma_start(xt[:], xf[i])
                        nc.sync.dma_start(yf[i], xt[:])
                elif mode == "dmabig":
                    for g in range(GROUPS_PER_CORE):
                        rows = slice(g * 128, (g + 1) * 128)
                        xb = xpool.tile([128, N_COLS], fmm, name="xb",
                                        bufs=2)
                        nc.sync.dma_start(xb[:], x[rows, :])
                        nc.sync.dma_start(yout[rows, :], xb[:])
                elif mode == "compute":
                    xc = cpool.tile([128, 1024], fmm, name="xc")
                    nc.vector.memset(xc[:], 1.0)
                    _emit_body(nc, tc, mybir, None, None, win_sb, wout_sb,
                               bias_sb, m_sb, sqb_sb, None, wpool, pypool,
                               popool, fmm, xc=xc)
                elif mode == "inpe":
                    # in-DMA + stage-1 MMs + PSUM evac only
                    for g in range(GROUPS_PER_CORE):
                        lhs1 = win_sb[:, g * 128:(g + 1) * 128]
                        for j in range(N_COLS // 1024):
                            xt = xpool.tile([128, 1024], fmm, name="xt")
                            nc.sync.dma_start(
                                xt[:], x[g * 128:(g + 1) * 128,
                                         j * 1024:(j + 1) * 1024])
                            py = pypool.tile([128, 1024], f32, name="py")
                            for h in range(2):
                                cs = slice(h * 512, (h + 1) * 512)
                                nc.tensor.matmul(py[:, cs], lhs1, xt[:, cs],
                                                 start=True, stop=True)
                            yt = wpool.tile([128, 1024], f32, name="yt")
                            nc.scalar.activation(
                                yt[:], py[:],
                                mybir.ActivationFunctionType.Identity,
                                bias=bias_sb[:, g:g + 1], scale=1.0)
                elif mode == "indep":
                    # compute from a memset tile + unconsumed in-DMAs:
                    # isolates DMA-write/engine contention from deps
                    xc = cpool.tile([128, 1024], fmm, name="xc")
                    nc.vector.memset(xc[:], 1.0)
                    for g in range(GROUPS_PER_CORE):
                        for j in range(N_COLS // 1024):
                            dummy = xpool.tile([128, 1024], fmm,
                                               name="dummy")
                            nc.sync.dma_start(
                                dummy[:],
                                x[g * 128:(g + 1) * 128,
                                  j * 1024:(j + 1) * 1024])
                    _emit_body(nc, tc, mybir, None, None, win_sb, wout_sb,
                               bias_sb, m_sb, sqb_sb, None, wpool, pypool,
                               popool, fmm, xc=xc, pipelined=pipelined,
                               odma=odma)
                elif mode == "noout":
                    _emit_body(nc, tc, mybir, x, None, win_sb, wout_sb,
                               bias_sb, m_sb, sqb_sb, xpool, wpool, pypool,
                               popool, fmm, pipelined=pipelined, odma=odma,
                               interpose=interpose, XW=xw)
                elif mode == "noin":
                    xc = cpool.tile([128, 1024], fmm, name="xc")
                    nc.vector.memset(xc[:], 1.0)
                    _emit_body(nc, tc, mybir, None, yout, win_sb, wout_sb,
                               bias_sb, m_sb, sqb_sb, None, wpool, pypool,
                               popool, fmm, xc=xc, pipelined=pipelined,
                               odma=odma)

    nc.compile()
    return nc


def _emit_body(nc, tc, mybir, x, yout, win_sb, wout_sb, bias_sb, m_sb, sqb_sb,
               xpool, wpool, pypool, popool, fmm, xc=None, pipelined=True,
               odma="sp", W=1024, interpose=False, XW=None):
    f32 = mybir.dt.float32
    AFT = mybir.ActivationFunctionType
    Alu = mybir.AluOpType
    n_wtiles = N_COLS // W
    if XW is None or XW < W:
        XW = W
    tiles_per_xw = XW // W

    # Software-pipelined: stage 2 of megatile k-1 is emitted after the
    # elementwise chain of megatile k, so PE never waits on z.
    tiles = [(g, j) for g in range(GROUPS_PER_CORE) for j in range(n_wtiles)]
    pending = None  # (g, j, zt)
    it = 0
    out_eng = {"sp": nc.sync, "act": nc.scalar, "pool": nc.gpsimd}[odma]

    def stage2(g, j, zt, it):
        lhs2 = wout_sb[:, g * 128:(g + 1) * 128]
        po = popool.tile([128, W], f32, name="po")
        for h in range(W // FREE):
            cs = slice(h * FREE, (h + 1) * FREE)
            nc.tensor.matmul(po[:, cs], lhs2, zt[:, cs],
                             start=True, stop=True)
        ot = wpool.tile([128, W], f32, name="ot")
        if it % 2 == 0:
            nc.vector.tensor_copy(ot[:], po[:])
        else:
            nc.scalar.copy(ot[:], po[:])
        if yout is not None:
            out_eng.dma_start(
                yout[g * 128:(g + 1) * 128, j * W:(j + 1) * W], ot[:])

    for (g, j) in tiles:
        rows = slice(g * 128, (g + 1) * 128)
        cols = slice(j * W, (j + 1) * W)
        lhs1 = win_sb[:, g * 128:(g + 1) * 128]
        bias_g = bias_sb[:, g:g + 1]
        if xc is not None:
            xt = xc
        else:
            if j % tiles_per_xw == 0:
                xt_wide = xpool.tile([128, XW], fmm, name="xt")
                nc.sync.dma_start(
                    xt_wide[:], x[rows, j * W:j * W + XW])
                if interpose:
                    xt2 = xpool.tile([128, XW], fmm, name="xt2")
                    nc.gpsimd.tensor_copy(xt2[:], xt_wide[:])
                    xt_wide = xt2
            off = (j % tiles_per_xw) * W
            xt = xt_wide[:, off:off + W]

        # stage 1: y' = diag(d).Min @ x + d*b   (one MM per PSUM bank)
        py = pypool.tile([128, W], f32, name="py")
        for h in range(W // FREE):
            cs = slice(h * FREE, (h + 1) * FREE)
            nc.tensor.matmul(py[:, cs], lhs1, xt[:, cs],
                             start=True, stop=True)
        yt = wpool.tile([128, W], f32, name="yt")
        nc.scalar.activation(yt[:], py[:], AFT.Identity,
                             bias=bias_g, scale=1.0)

        # t2 = y'^2 ; s = sqrt(m*t2 + (m*0.05)^2) ; z = y' + s
        tt = wpool.tile([128, W], f32, name="tt")
        nc.vector.tensor_tensor(tt[:], yt[:], yt[:], Alu.mult)
        st = wpool.tile([128, W], f32, name="st")
        nc.scalar.activation(st[:], tt[:], AFT.Sqrt,
                             bias=sqb_sb[:, 0:1],
                             scale=m_sb[:, 0:1])
        zt = wpool.tile([128, W], fmm, name="zt")
        nc.vector.tensor_tensor(zt[:], yt[:], st[:], Alu.add)

        if not pipelined:
            stage2(g, j, zt, it)
            it += 1
        else:
            if pending is not None:
                stage2(*pending, it)
                it += 1
            pending = (g, j, zt)

    if pending is not None:
        stage2(*pending, it)


def _get_program():
    if "nc" not in _PROGRAM_CACHE:
        _PROGRAM_CACHE["nc"] = _build_program()
    return _PROGRAM_CACHE["nc"]


def kernel(data, angles, biases, indices_in, idx_out, _return_results=False):
    from concourse import bass_utils

    data = np.asarray(data)
    x_full = np.ascontiguousarray(
        np.asarray(data, np.float32)[np.asarray(indices_in)]
    )
    weights = _host_weights(angles, biases)
    in_maps = []
    for c in range(N_CORES):
        im = dict(weights[c])
        im["x"] = np.ascontiguousarray(
            x_full[c * ROWS_PER_CORE:(c + 1) * ROWS_PER_CORE]
        )
        in_maps.append(im)

    nc = _get_program()
    res = bass_utils.run_bass_kernel_spmd(nc, in_maps,
                                          core_ids=list(range(N_CORES)))
    y = np.concatenate([res.results[c]["yout"] for c in range(N_CORES)], axis=0)
    out = np.array(data, copy=True)
    out[np.asarray(idx_out)] = y
    if _return_results:
        return out, res
    return out



# revision 7
# speedup vs baseline: 2.7255x; 2.7255x over previous
"""Trainium2 Bass kernel for the ButterflyModule problem (packed-bf16 v3).

Semantics (N=4096 rows, B=8192 cols):
  x = data[indices_in]
  4 Givens-rotation butterfly layers (strides 1,2,4,8 within 16-row blocks)
  bias + smooth-ReLU on rows with (row%16)<8
  4 more butterfly layers (strides 1,2,4,8)
  out = data with rows idx_out replaced by the result

Math: per 128-row group, with W1 = diag(d).Min (block-diag 16x16 composed,
act rows scaled by 0.5), Wo = Mout block-diag, A = act rows (row%16<8),
b' = 0.5*bias on act rows:

  y''     = W1 @ x + b'
  s       = sqrt(m*(y'')^2 + (0.05)^2 m)    (nonzero only on act rows)
  out     = Wo @ (y'' + s) = (Wo@W1) @ x + Wo[:,A] @ s[A] + Wo[:,A] @ b'[A]
          = Cfull @ x + WoutA @ s_A + c2

Device pipeline per 2048-col unit (all matmul I/O in bf16, PSUM f32):
  pact[0:64]   = Wact @ x[:, 0:1024]      (Wact = W1[A,:], packed 2 halves)
  pact[64:128] = Wact @ x[:, 1024:2048]
  t = Square(pact + b'_A)   (ACT, bf16)
  s = Sqrt(t + 0.0025)      (ACT, bf16)
  po = Cfull @ x_half + WoutA @ s_half    (PE accumulate)
  ot = po + c2              (DVE tensor_scalar, bf16)
  DMA out.

The 2e-2 rel-err budget easily covers bf16 I/O (measured 4.9e-3 in host sim).
Rows are sharded across the 8 cores (512 rows each); rotations never cross
16-row block boundaries so there is no cross-core communication.
"""

import sys

if "/opt/trn_rl_repo" not in sys.path:
    sys.path.insert(0, "/opt/trn_rl_repo")

import numpy as np
import ml_dtypes

BF16 = ml_dtypes.bfloat16

N_ROWS = 4096
N_COLS = 8192
COL_BLOCK = 16
NUM_ACT = 8
CURVATURE = 0.1
N_CORES = 8
ROWS_PER_CORE = N_ROWS // N_CORES          # 512
GROUPS_PER_CORE = ROWS_PER_CORE // 128     # 4
W = 2048                                   # unit width (cols per pipeline unit)
HALF = W // 2                              # per-PSUM-tile free dim
N_UNITS = N_COLS // W                      # 4 per group

_PROGRAM_CACHE = {}


def _butterfly_mats(angles64):
    """Compose butterfly layers into per-block 16x16 matrices.

    angles64: [8, 2048] float64.  Returns (Min, Mout) each [256, 16, 16],
    where layer l uses stride 1<<(l%4) and block b uses angles[l, 8b:8b+8]
    ordered by the low row index within the block.
    """
    nb = N_ROWS // COL_BLOCK

    def accum(l0, l1):
        G = np.broadcast_to(np.eye(COL_BLOCK), (nb, COL_BLOCK, COL_BLOCK)).copy()
        for l in range(l0, l1):
            stride = 1 << (l % 4)
            offs = [o for o in range(COL_BLOCK) if (o & stride) == 0]
            a = angles64[l].reshape(nb, NUM_ACT)
            c = np.cos(a)
            s = np.sin(a)
            for k, o in enumerate(offs):
                gl = G[:, o, :].copy()
                gh = G[:, o + stride, :].copy()
                G[:, o, :] = c[:, k, None] * gl + s[:, k, None] * gh
                G[:, o + stride, :] = -s[:, k, None] * gl + c[:, k, None] * gh
        return G

    return accum(0, 4), accum(4, 8)


def _host_weights(angles, biases):
    """Build per-core weight tensors for the v3 device kernel."""
    ang64 = np.asarray(angles, np.float64)
    b64 = np.asarray(biases, np.float64)
    Min, Mout = _butterfly_mats(ang64)

    off16 = np.arange(COL_BLOCK)
    d16 = np.where(off16 < NUM_ACT, 0.5, 1.0)
    Minp = Min * d16[None, :, None]                  # y'' rows pre-scaled

    offs = np.arange(128) % COL_BLOCK
    A = np.nonzero(offs < NUM_ACT)[0]                # 64 act rows per group

    n_groups = N_ROWS // 128
    wactT = np.zeros((n_groups, 128, 64))
    woutaT = np.zeros((n_groups, 64, 128))
    cfullT = np.zeros((n_groups, 128, 128))
    biassq = np.zeros((n_groups, 128))
    c2 = np.zeros((n_groups, 128))

    for g in range(n_groups):
        W1 = np.zeros((128, 128))
        Wo = np.zeros((128, 128))
        for i in range(8):
            W1[i*16:(i+1)*16, i*16:(i+1)*16] = Minp[g*8+i]
            Wo[i*16:(i+1)*16, i*16:(i+1)*16] = Mout[g*8+i]
        Wact = W1[A, :]                   # [64,128]
        WoutA = Wo[:, A]                  # [128,64]
        Cfull = Wo @ W1                   # [128,128]
        bpp = np.zeros(128)
        for i in range(8):
            blk = g * 8 + i
            bpp[i*16:i*16+8] = 0.5 * b64[blk*8:(blk+1)*8]
        b_act = bpp[A]                    # [64]
        wactT[g] = Wact.T
        woutaT[g] = WoutA.T
        cfullT[g] = Cfull.T
        biassq[g] = np.concatenate([b_act, b_act])   # both packed halves
        c2[g] = WoutA @ b_act

    per_core = []
    for c in range(N_CORES):
        gs = slice(c * GROUPS_PER_CORE, (c + 1) * GROUPS_PER_CORE)
        # [128, G*64] / [64, G*128] / [128, G*128] with group-major columns
        wact_d = wactT[gs].transpose(1, 0, 2).reshape(128, -1)
        wouta_d = woutaT[gs].transpose(1, 0, 2).reshape(64, -1)
        wouta_d = np.concatenate([wouta_d, wouta_d], axis=0)   # both halves
        cfull_d = cfullT[gs].transpose(1, 0, 2).reshape(128, -1)
        biassq_d = biassq[gs].T                      # [128, G]
        c2_d = c2[gs].T                              # [128, G]
        per_core.append({
            "wact": np.ascontiguousarray(wact_d, dtype=BF16),
            "wouta": np.ascontiguousarray(wouta_d, dtype=BF16),
            "cfull": np.ascontiguousarray(cfull_d, dtype=BF16),
            "biassq": np.ascontiguousarray(biassq_d, dtype=np.float32),
            "c2t": np.ascontiguousarray(c2_d, dtype=np.float32),
        })
    return per_core


def _build_program(reps=None, mode=None, xbufs=None, wbufs=None, obufs=None,
                   odma=None):
    import os
    import contextlib

    import concourse.bacc as bacc
    import concourse.mybir as mybir
    from concourse.tile import TileContext

    f32 = mybir.dt.float32
    bf16 = mybir.dt.bfloat16
    AFT = mybir.ActivationFunctionType
    Alu = mybir.AluOpType
    if reps is None:
        reps = int(os.environ.get("BUTTERFLY_REPS", "1"))
    if mode is None:
        mode = os.environ.get("BUTTERFLY_MODE", "full")  # full|dma
    if xbufs is None:
        xbufs = int(os.environ.get("BUTTERFLY_XBUFS", "3"))
    if wbufs is None:
        wbufs = int(os.environ.get("BUTTERFLY_WBUFS", "4"))
    if obufs is None:
        obufs = int(os.environ.get("BUTTERFLY_OBUFS", "3"))
    if odma is None:
        odma = os.environ.get("BUTTERFLY_ODMA", "sp")  # sp | act

    nc = bacc.Bacc("TRN2", target_bir_lowering=False)
    x = nc.dram_tensor("x", [ROWS_PER_CORE, N_COLS], bf16, kind="ExternalInput")
    wact = nc.dram_tensor("wact", [128, GROUPS_PER_CORE * 64], bf16,
                          kind="ExternalInput")
    wouta = nc.dram_tensor("wouta", [128, GROUPS_PER_CORE * 128], bf16,
                           kind="ExternalInput")
    cfull = nc.dram_tensor("cfull", [128, GROUPS_PER_CORE * 128], bf16,
                           kind="ExternalInput")
    biassq = nc.dram_tensor("biassq", [128, GROUPS_PER_CORE], f32,
                            kind="ExternalInput")
    c2t = nc.dram_tensor("c2t", [128, GROUPS_PER_CORE], f32,
                         kind="ExternalInput")
    yout = nc.dram_tensor("yout", [ROWS_PER_CORE, N_COLS], bf16,
                          kind="ExternalOutput")

    with TileContext(nc) as tc:
        with (
            tc.tile_pool(name="consts", bufs=1) as cpool,
            tc.tile_pool(name="xin", bufs=xbufs) as xpool,
            tc.tile_pool(name="work", bufs=wbufs) as wpool,
            tc.tile_pool(name="outb", bufs=obufs) as opool,
            tc.tile_pool(name="psum_y", bufs=2, space="PSUM") as pypool,
            tc.tile_pool(name="psum_o", bufs=2, space="PSUM") as popool,
        ):
            wact_sb = cpool.tile([128, GROUPS_PER_CORE * 64], bf16)
            wouta_sb = cpool.tile([128, GROUPS_PER_CORE * 128], bf16)
            cfull_sb = cpool.tile([128, GROUPS_PER_CORE * 128], bf16)
            biassq_sb = cpool.tile([128, GROUPS_PER_CORE], f32)
            c2_sb = cpool.tile([128, GROUPS_PER_CORE], f32)
            sqb_sb = cpool.tile([128, 1], f32)
            nc.vector.memset(sqb_sb[:], (0.5 * CURVATURE) ** 2)
            nc.sync.dma_start(wact_sb[:], wact[:])
            nc.sync.dma_start(wouta_sb[:], wouta[:])
            nc.sync.dma_start(cfull_sb[:], cfull[:])
            nc.sync.dma_start(biassq_sb[:], biassq[:])
            nc.sync.dma_start(c2_sb[:], c2t[:])

            out_eng = {"sp": nc.sync, "act": nc.scalar}[odma]

            loop_cm = (tc.For_i(0, reps, 1) if reps > 1
                       else contextlib.nullcontext())
            with loop_cm:
                if mode == "dma":
                    # pure DMA round trip at bf16 (roofline probe)
                    for g in range(GROUPS_PER_CORE):
                        rows = slice(g * 128, (g + 1) * 128)
                        for j in range(N_UNITS):
                            cols = slice(j * W, (j + 1) * W)
                            xt = xpool.tile([128, W], bf16, name="xt")
                            nc.sync.dma_start(xt[:], x[rows, cols])
                            out_eng.dma_start(yout[rows, cols], xt[:])
                else:
                    _emit_body(nc, mybir, x, yout, wact_sb, wouta_sb,
                               cfull_sb, biassq_sb, c2_sb, sqb_sb, xpool,
                               wpool, opool, pypool, popool, out_eng)

    nc.compile()
    return nc


def _emit_body(nc, mybir, x, yout, wact_sb, wouta_sb, cfull_sb, biassq_sb,
               c2_sb, sqb_sb, xpool, wpool, opool, pypool, popool, out_eng):
    f32 = mybir.dt.float32
    bf16 = mybir.dt.bfloat16
    AFT = mybir.ActivationFunctionType
    Alu = mybir.AluOpType

    units = [(g, j) for g in range(GROUPS_PER_CORE) for j in range(N_UNITS)]
    pending = None   # (g, j, xt, s)

    def stage2(g, j, xt, s):
        cfull_g = cfull_sb[:, g * 128:(g + 1) * 128]
        wouta_g = wouta_sb[:, g * 128:(g + 1) * 128]
        c2_g = c2_sb[:, g:g + 1]
        rows = slice(g * 128, (g + 1) * 128)
        ot = opool.tile([128, W], bf16, name="ot")
        for h in range(2):
            cs = slice(h * HALF, (h + 1) * HALF)
            ps = slice(h * 64, (h + 1) * 64)
            po = popool.tile([128, HALF], f32, name="po")
            for q in range(HALF // 512):
                qs = slice(q * 512, (q + 1) * 512)
                qx = slice(cs.start + q * 512, cs.start + (q + 1) * 512)
                nc.tensor.matmul(po[:, qs], cfull_g, xt[:, qx],
                                 start=True, stop=False)
                nc.tensor.matmul(po[:, qs], wouta_g[ps.start:ps.stop, :],
                                 s[ps, qs], start=False, stop=True,
                                 skip_group_check=True)
            nc.vector.tensor_scalar(
                out=ot[:, cs], in0=po[:], scalar1=c2_g, scalar2=None,
                op0=Alu.add)
        out_eng.dma_start(yout[rows, j * W:(j + 1) * W], ot[:])

    for (g, j) in units:
        rows = slice(g * 128, (g + 1) * 128)
        wact_g = wact_sb[:, g * 64:(g + 1) * 64]
        bsq_g = biassq_sb[:, g:g + 1]

        xt = xpool.tile([128, W], bf16, name="xt")
        nc.sync.dma_start(xt[:], x[rows, j * W:(j + 1) * W])

        pact = pypool.tile([128, HALF], f32, name="pact")
        for h in range(2):
            for q in range(HALF // 512):
                qs = slice(q * 512, (q + 1) * 512)
                qx = slice(h * HALF + q * 512, h * HALF + (q + 1) * 512)
                nc.tensor.matmul(pact[h * 64:(h + 1) * 64, qs], wact_g,
                                 xt[:, qx], start=True, stop=True)

        t = wpool.tile([128, HALF], bf16, name="t")
        nc.scalar.activation(t[:], pact[:], AFT.Square, bias=bsq_g, scale=1.0)
        s = wpool.tile([128, HALF], bf16, name="s")
        nc.scalar.activation(s[:], t[:], AFT.Sqrt,
                             bias=sqb_sb[:, 0:1], scale=1.0)

        if pending is not None:
            stage2(*pending)
        pending = (g, j, xt, s)

    if pending is not None:
        stage2(*pending)


def _get_program():
    if "nc" not in _PROGRAM_CACHE:
        _PROGRAM_CACHE["nc"] = _build_program()
    return _PROGRAM_CACHE["nc"]


def kernel(data, angles, biases, indices_in, idx_out, _return_results=False):
    from concourse import bass_utils

    data = np.asarray(data)
    x_full = np.asarray(data, np.float32)[np.asarray(indices_in)]
    x_bf = np.ascontiguousarray(x_full.astype(BF16))
    weights = _host_weights(angles, biases)
    in_maps = []
    for c in range(N_CORES):
        im = dict(weights[c])
        im["x"] = np.ascontiguousarray(
            x_bf[c * ROWS_PER_CORE:(c + 1) * ROWS_PER_CORE]
        )
        in_maps.append(im)

    nc = _get_program()
    res = bass_utils.run_bass_kernel_spmd(nc, in_maps,
                                          core_ids=list(range(N_CORES)))
    y = np.concatenate(
        [np.asarray(res.results[c]["yout"]) for c in range(N_CORES)], axis=0
    ).astype(np.float32)
    out = np.array(data, copy=True)
    out[np.asarray(idx_out)] = y
    if _return_results:
        return out, res
    return out
